# revision 25
# baseline (speedup 1.0000x reference)
"""Trainium2 Bass kernel for nn_Decoder_46042049413334.

Buggy 2-layer LSTM decoder with attention (B=32, T=64, S=128, D=512).

v2: col-tiled recurrence. Per step, the gates psum is one bank [128, 512]
laid out as partition block c = batch rows (4 of 32) for d-slice c, free
cols [i | f | g | o] x 128. The four PE column-groups stream four
different 512-wide W_hh column slices concurrently (tile_position), so a
full W_hh pass costs ~4x less PE wall time than a single-stream matmul.
g-columns are pre-scaled by 2 so one 384-wide sigmoid gives sig(i),
sig(f), sig(2g) and tanh(g) = 2*sig(2g) - 1 comes from cheap DVE ops.
h2 = sig(o)*tanh(c2) is deferred: sig(o) and sig(2*c2) are stored per
step and h2 is computed batched between passes. State transposes
(c2 [b, d] -> c2T [d, b]) are 4 tiny col-tiled identity matmuls.

Row ordering is b-major everywhere: row r = b_local*T + t.
"""
import numpy as np
import ml_dtypes
from contextlib import ExitStack

import concourse.bass as bass
import concourse.bacc as bacc
import concourse.tile as tile
from concourse import mybir, masks
from concourse.bass_utils import run_bass_kernel_spmd

F32 = mybir.dt.float32
BF16 = mybir.dt.bfloat16
AF = mybir.ActivationFunctionType
ALU = mybir.AluOpType
NPBF = ml_dtypes.bfloat16

B, T, S, D, L, V = 32, 64, 128, 512, 2, 32000
G = 4 * D        # 2048
DS = 2 * D       # 1024
NCORES = 8
BS = B // NCORES  # 4
R = BS * T        # 256 rows per core


# ---------------------------------------------------------------- host side

def _gate_perm():
    """New gate col n' = 512c + 128s + dd  <-  orig = 512s + 128c + dd.

    Per-128-d-slice c the col layout is [i | f | g | o] (s = 0..3), which
    keeps sig-able gates in cols [0:384] of each group's 512-wide slice.
    """
    idx = np.arange(G)
    c, rem = idx // 512, idx % 512
    s, dd = rem // 128, rem % 128
    return 512 * s + 128 * c + dd  # orig index for each new position


def _g_scale():
    """Per-new-col scale: 2.0 on g columns (s==2), else 1.0."""
    idx = np.arange(G)
    s = (idx % 512) // 128
    return np.where(s == 2, 2.0, 1.0)


def host_prep(inputs):
    perm = _gate_perm()
    gsc = _g_scale()
    tokens = np.asarray(inputs["prev_tgt_tokens"])
    embed = np.asarray(inputs["embed"], dtype=np.float32)
    enc = np.asarray(inputs["encoder_out"], dtype=np.float32)
    mask = np.asarray(inputs["src_mask"])
    hid = np.asarray(inputs["hiddens"], dtype=np.float32)
    cells = np.asarray(inputs["cells"], dtype=np.float32)
    W_ih = np.asarray(inputs["W_ih"], dtype=np.float32)
    W_hh = np.asarray(inputs["W_hh"], dtype=np.float32)
    b_ih = np.asarray(inputs["b_ih"], dtype=np.float32)
    b_hh = np.asarray(inputs["b_hh"], dtype=np.float32)
    W_in = np.asarray(inputs["W_in"], dtype=np.float32)
    b_in = np.asarray(inputs["b_in"], dtype=np.float32)
    W_out = np.asarray(inputs["W_out"], dtype=np.float32)
    b_out = np.asarray(inputs["b_out"], dtype=np.float32)

    def bf(x):
        return np.ascontiguousarray(x, dtype=NPBF)

    WIH = []
    WHH = []
    for l in range(L):
        wihT = W_ih[l].T[:, perm] * gsc[None, :]
        biasrow = ((b_ih[l] + b_hh[l])[perm] * gsc)[None, :]
        WIH.append(bf(np.concatenate([wihT, biasrow], 0)))   # [513, 2048]
        WHH.append(bf(W_hh[l].T[:, perm] * gsc[None, :]))    # [512, 2048]
    WINT = bf(W_in.T)                                        # [512, 1024]
    WOUTT = bf(np.concatenate([W_out.T, b_out[None, :]], 0))  # [1537, 512]

    # I16[k, 4j+b] = 1 iff k == 32j + b
    I16 = np.zeros((128, 16), np.float32)
    for j in range(4):
        for b in range(4):
            I16[32 * j + b, 4 * j + b] = 1.0
    # IWC[c, k, 32*tl + m] = 1 iff m < 4 and k == 32c + 4*tl + m
    # (xp identity: selects step tl's batch rows out of an 8-step oct tile,
    #  writing zeros to out partitions 4..31 so psum is fully initialized)
    IWC = np.zeros((4, 128, 256), np.float32)
    for c in range(4):
        for tl in range(8):
            for mm in range(4):
                IWC[c, 32 * c + 4 * tl + mm, 32 * tl + mm] = 1.0
    IWC = bf(IWC)

    in_maps = []
    for core in range(NCORES):
        bsl = slice(core * BS, (core + 1) * BS)
        xe = embed[tokens[bsl]]                              # [BS, T, D]
        # t-major rows: r = 4t + b
        Xaug = np.concatenate(
            [xe.transpose(1, 0, 2).reshape(R, D),
             np.ones((R, 1), np.float32)], axis=1)
        XT0 = bf(Xaug.T)                                     # [513, 256]
        enc_c = np.ascontiguousarray(enc[bsl])               # [BS, 128, 1024]
        encT_c = np.swapaxes(enc_c, 1, 2)                    # [BS, 1024, 128]
        offs = np.einsum("bsd,d->bs", enc_c, b_in) + np.where(mask[bsl], -1e9, 0.0)
        offs_rep = np.ascontiguousarray(
            np.broadcast_to(offs[:, None, :], (BS, T, S)), dtype=np.float32)
        # c2T0[p, 4ko+b] = hiddens[l, b, 128ko+p]
        hl = hid[:, bsl]                                     # [L, 4, 512]
        c2t0 = np.zeros((L, 128, 16), np.float32)
        for ko in range(4):
            for b in range(4):
                c2t0[:, :, 4 * ko + b] = hl[:, b, 128 * ko:128 * ko + 128]
        # c0grid[32q+b, dd] = cells[l, b, 128q+dd]
        cl = cells[:, bsl]                                   # [L, 4, 512]
        c0g = np.zeros((L, 128, 128), np.float32)
        for q in range(4):
            for b in range(4):
                c0g[:, 32 * q + b, :] = cl[:, b, 128 * q:128 * q + 128]
        in_maps.append({
            "xt0": XT0,
            "wih0": WIH[0], "whh0": WHH[0],
            "wih1": WIH[1], "whh1": WHH[1],
            "wint": WINT, "woutt": WOUTT,
            "enc": bf(enc_c), "enct": bf(encT_c), "offs": offs_rep,
            "c2t0": bf(c2t0), "c0g": bf(c0g), "i16": I16, "iw": IWC,
            "ones1": np.ones((1, R), NPBF),
            "ones128": np.ones((128, 128), np.float32),
            "id4": np.eye(BS, dtype=NPBF),
        })
    return in_maps


# ------------------------------------------------------------- device build

def build_program():
    nc = bacc.Bacc("TRN2", target_bir_lowering=False, debug=False)

    XT0 = nc.dram_tensor("xt0", [513, R], BF16, kind="ExternalInput")
    WIH0 = nc.dram_tensor("wih0", [513, G], BF16, kind="ExternalInput")
    WHH0 = nc.dram_tensor("whh0", [D, G], BF16, kind="ExternalInput")
    WIH1 = nc.dram_tensor("wih1", [513, G], BF16, kind="ExternalInput")
    WHH1 = nc.dram_tensor("whh1", [D, G], BF16, kind="ExternalInput")
    WINT = nc.dram_tensor("wint", [D, DS], BF16, kind="ExternalInput")
    WOUTT = nc.dram_tensor("woutt", [DS + D + 1, D], BF16, kind="ExternalInput")
    ENC = nc.dram_tensor("enc", [BS, S, DS], BF16, kind="ExternalInput")
    ENCT = nc.dram_tensor("enct", [BS, DS, S], BF16, kind="ExternalInput")
    OFFS = nc.dram_tensor("offs", [BS, T, S], F32, kind="ExternalInput")
    C2T0 = nc.dram_tensor("c2t0", [L, 128, 16], BF16, kind="ExternalInput")
    C0G = nc.dram_tensor("c0g", [L, 128, 128], BF16, kind="ExternalInput")
    I16T = nc.dram_tensor("i16", [128, 16], F32, kind="ExternalInput")
    IWT = nc.dram_tensor("iw", [4, 128, 256], BF16, kind="ExternalInput")
    ONES1 = nc.dram_tensor("ones1", [1, R], BF16, kind="ExternalInput")
    ONES128 = nc.dram_tensor("ones128", [128, 128], F32, kind="ExternalInput")
    ID4 = nc.dram_tensor("id4", [BS, BS], BF16, kind="ExternalInput")
    OUT = nc.dram_tensor("out", [BS, T, D], F32, kind="ExternalOutput")

    XPAD0 = nc.dram_tensor("xpad0", [T // 8, 128, 512], BF16, kind="Internal")
    XPAD1 = nc.dram_tensor("xpad1", [T // 8, 128, 512], BF16, kind="Internal")
    SO4 = nc.dram_tensor("so4", [L, T, 128, 128], F32, kind="Internal")
    S2C4 = nc.dram_tensor("s2c4", [L, T, 128, 128], F32, kind="Internal")
    H2S = nc.dram_tensor("h2s", [L, T, BS, D], F32, kind="Internal")

    with tile.TileContext(nc) as tc, ExitStack() as ctx:
        cpool = ctx.enter_context(tc.tile_pool(name="const", bufs=1))
        ident = cpool.tile([128, 128], F32)
        masks.make_identity(nc, ident[:])
        ones = cpool.tile([1, R], BF16)
        nc.sync.dma_start(ones[:], ONES1.ap())
        ones128 = cpool.tile([128, 128], F32)
        nc.sync.dma_start(ones128[:], ONES128.ap())
        i4r = cpool.tile([BS, BS], BF16)
        nc.sync.dma_start(i4r[:], ID4.ap())
        i16 = cpool.tile([128, 16], F32)
        nc.sync.dma_start(i16[:], I16T.ap())
        iw = [cpool.tile([128, 256], BF16, tag=f"iw{c}", name=f"iw{c}")
              for c in range(4)]
        for c in range(4):
            nc.sync.dma_start(iw[c][:], IWT.ap()[c])

        psp = ctx.enter_context(tc.tile_pool(name="ps", bufs=1, space="PSUM"))

        def gtile(idx, shape):
            return psp.tile(shape, F32, tag=f"g{idx}", name=f"g{idx}", bufs=1)

        def batched_xpart(wpool, lhs_tiles, W_dram, XPAD_dram):
            """xpart = lhsT.T @ W  -> XPAD_dram [T/8, 128, 512] oct layout.

            Out rows are t-major (r = 4t + b); oct th holds partition
            p = 32c + 4*(t%8) + b for permuted col slice c, so the store
            is a plain 3D slice.
            """
            wt = [wpool.tile([128, G], BF16, tag=f"wk{k}", name=f"wk{k}")
                  for k in range(4)]
            wt.append(wpool.tile([1, G], BF16, tag="wk4", name="wk4"))
            for k in range(4):
                nc.sync.dma_start(wt[k][:], W_dram.ap()[128 * k:128 * (k + 1), :])
            nc.sync.dma_start(wt[4][:], W_dram.ap()[512:513, :])
            for mc in range(2):
                for nb in range(4):
                    ps = gtile(nb, [128, 512])
                    for k in range(5):
                        nc.tensor.matmul(
                            ps[:],
                            lhs_tiles[k][:, 128 * mc:128 * (mc + 1)],
                            wt[k][:, 512 * nb:512 * (nb + 1)],
                            start=(k == 0), stop=(k == 4))
                    sb = wpool.tile([128, 512], BF16, tag=f"stg{nb}",
                                    name=f"stg{nb}")
                    nc.scalar.copy(sb[:], ps[:])
                    # sb rows r-128mc = (th'(4), tl(8), b(4)); p = 32nb+4tl+b
                    nc.sync.dma_start(
                        XPAD_dram.ap()[4 * mc:4 * mc + 4,
                                       32 * nb:32 * nb + 32, :],
                        sb[:])

        # hoisted W_hh loads for both layers (overlap with phase A)
        wbpool = ctx.enter_context(tc.tile_pool(name="wb", bufs=1))
        whh_all = {}
        for l, Wd in ((0, WHH0), (1, WHH1)):
            tiles = [wbpool.tile([128, G], BF16, tag=f"whh{l}k{k}",
                                 name=f"whh{l}k{k}") for k in range(4)]
            for k in range(4):
                nc.sync.dma_start(
                    tiles[k][:], Wd.ap()[128 * k:128 * (k + 1), :])
            whh_all[l] = tiles

        # ---------------- Phase A: xpart0 ----------------
        with tc.tile_pool(name="pa", bufs=1) as pa:
            xt = [pa.tile([128, R], BF16, tag=f"xt{k}", name=f"xt{k}")
                  for k in range(4)]
            xt.append(pa.tile([1, R], BF16, tag="xt4", name="xt4"))
            for k in range(4):
                nc.sync.dma_start(xt[k][:], XT0.ap()[128 * k:128 * (k + 1), :])
            nc.sync.dma_start(xt[4][:], XT0.ap()[512:513, :])
            batched_xpart(pa, xt, WIH0, XPAD0)

        # ---------------- Recurrence passes ----------------
        def recur(l, XPAD):
            with tc.tile_pool(name=f"rr{l}", bufs=1) as rp:
                c0g = rp.tile([128, 128], BF16, tag="c0g", name="c0g")
                nc.sync.dma_start(c0g[:], C0G.ap()[l])
                c2t = rp.tile([128, 16], BF16, tag="c2t", name="c2t", bufs=2)
                nc.sync.dma_start(c2t[:], C2T0.ap()[l])

                xp_tiles = {}

                def xp_load(g8):
                    xp = rp.tile([128, 512], BF16, tag="xp", name="xp", bufs=3)
                    nc.sync.dma_start(xp[:], XPAD.ap()[g8])
                    xp_tiles[g8] = xp

                gates = {}

                def xp_mm(t):
                    ps = psp.tile([128, 512], F32, tag="rg", name="rg", bufs=2)
                    gates[t] = ps
                    tl = t % 8
                    for c in range(4):
                        nc.tensor.matmul(
                            ps[32 * c:32 * c + 32, :],
                            iw[c][:, 32 * tl:32 * tl + 32],
                            xp_tiles[t // 8][:],
                            start=True, stop=False,
                            tile_position=(0, 32 * c), skip_group_check=True)

                xp_load(0)
                xp_load(1)
                xp_mm(0)
                xp_mm(1)

                for t in range(T):
                    ps = gates.pop(t)
                    whh = whh_all[l]
                    for ko in range(4):
                        for c in range(4):
                            nc.tensor.matmul(
                                ps[32 * c:32 * c + 4, :],
                                c2t[:, 4 * ko:4 * ko + 4],
                                whh[ko][:, 512 * c:512 * (c + 1)],
                                start=False, stop=(ko == 3),
                                tile_position=(0, 32 * c),
                                skip_group_check=True)
                    # S = sig([i | f | 2g])
                    sg = rp.tile([128, 384], F32, tag="sg", name="sg", bufs=2)
                    nc.scalar.activation(sg[:], ps[:, 0:384], AF.Sigmoid)
                    # sig(o) store (deferred h2)
                    so = rp.tile([128, 128], F32, tag="so", name="so", bufs=2)
                    nc.scalar.activation(so[:], ps[:, 384:512], AF.Sigmoid)
                    nc.sync.dma_start(SO4.ap()[l, t], so[:])
                    # c2 = sig(f)*c0 + sig(i)*(2*sig(2g) - 1)
                    u = rp.tile([128, 128], F32, tag="u", name="u", bufs=2)
                    nc.vector.scalar_tensor_tensor(
                        u[:], sg[:, 256:384], 2.0, sg[:, 0:128],
                        op0=ALU.mult, op1=ALU.mult)
                    v = rp.tile([128, 128], F32, tag="v", name="v", bufs=2)
                    nc.gpsimd.tensor_mul(v[:], sg[:, 128:256], c0g[:])
                    w = rp.tile([128, 128], F32, tag="w", name="w", bufs=2)
                    nc.vector.tensor_sub(w[:], u[:], sg[:, 0:128])
                    c2g = rp.tile([128, 128], F32, tag="c2g", name="c2g",
                                  bufs=2)
                    nc.vector.tensor_add(c2g[:], w[:], v[:])
                    # sig(2*c2) store (deferred h2)
                    s2c = rp.tile([128, 128], F32, tag="s2c", name="s2c",
                                  bufs=2)
                    nc.scalar.activation(s2c[:], c2g[:], AF.Sigmoid, scale=2.0)
                    nc.sync.dma_start(S2C4.ap()[l, t], s2c[:])
                    if t < T - 1:
                        # c2T via 4 col-tiled identity matmuls
                        tps = psp.tile([128, 16], F32, tag="rt", name="rt",
                                       bufs=1)
                        for cp in range(4):
                            nc.tensor.matmul(
                                tps[32 * cp:32 * cp + 32, :],
                                c2g[:, 32 * cp:32 * cp + 32], i16[:],
                                start=True, stop=True,
                                tile_position=(0, 32 * cp),
                                skip_group_check=True)
                        c2t = rp.tile([128, 16], BF16, tag="c2t", name="c2t",
                                      bufs=2)
                        nc.vector.tensor_copy(c2t[:], tps[:])
                        if t + 2 < T:
                            xp_mm(t + 2)
                        if (t + 2) % 8 == 0 and t + 2 < T - 8:
                            xp_load((t + 2) // 8 + 1)

        def h2_batch(l):
            """H2S[l] rows = sig(o) * (2*sig(2*c2) - 1), batched.

            SBUF tiles hold 8 steps: partition = 32c + 4t' + b (same oct
            grid as xp), so per-c loads/stores are 3D APs.
            """
            with tc.tile_pool(name=f"hb{l}", bufs=1) as hp:
                for ch in range(8):
                    t0 = 8 * ch
                    so8 = hp.tile([128, 128], F32, tag="so8", name="so8",
                                  bufs=2)
                    sc8 = hp.tile([128, 128], F32, tag="sc8", name="sc8",
                                  bufs=2)
                    for c in range(4):
                        nc.sync.dma_start(
                            so8[32 * c:32 * c + 32, :],
                            SO4.ap()[l, t0:t0 + 8, 32 * c:32 * c + 4, :])
                        nc.sync.dma_start(
                            sc8[32 * c:32 * c + 32, :],
                            S2C4.ap()[l, t0:t0 + 8, 32 * c:32 * c + 4, :])
                    th = hp.tile([128, 128], F32, tag="th", name="th", bufs=2)
                    nc.vector.scalar_tensor_tensor(
                        th[:], sc8[:], 2.0, ones128[:],
                        op0=ALU.mult, op1=ALU.subtract)
                    h2t = hp.tile([128, 128], F32, tag="h2t", name="h2t",
                                  bufs=2)
                    eng = nc.vector if ch % 2 == 0 else nc.gpsimd
                    eng.tensor_mul(h2t[:], th[:], so8[:])
                    for c in range(4):
                        nc.sync.dma_start(
                            H2S.ap()[l, t0:t0 + 8, :,
                                     128 * c:128 * c + 128],
                            h2t[32 * c:32 * c + 32, :])

        recur(0, XPAD0)
        h2_batch(0)

        # ---------------- Phase C: xpart1 from h2_0 ----------------
        def rows_from_stores(pool, l, tagpfx, tmajor):
            """Two [128, 512] row tiles of h2: rows r = 4t+b (tmajor) or
            r = 64b+t (else)."""
            if tmajor:
                flat = H2S.ap()[l].rearrange("t b d -> (t b) d")
            else:
                flat = H2S.ap()[l].rearrange("t b d -> b t d")
            outt = []
            for mc in range(2):
                h2 = pool.tile([128, D], F32, tag=f"{tagpfx}h{mc}",
                               name=f"{tagpfx}h{mc}")
                if tmajor:
                    nc.sync.dma_start(h2[:], flat[128 * mc:128 * (mc + 1), :])
                else:
                    nc.sync.dma_start(h2[:], flat[2 * mc:2 * mc + 2])
                outt.append(h2)
            return outt

        def transpose_rows(pool, rows, tagpfx):
            tT = [pool.tile([128, R], BF16, tag=f"{tagpfx}T{k}",
                            name=f"{tagpfx}T{k}") for k in range(4)]
            for mc in range(2):
                for k in range(4):
                    tp = psp.tile([128, 128], F32, tag="tp0", name="tp0")
                    nc.tensor.transpose(
                        tp[:], rows[mc][:, 128 * k:128 * (k + 1)], ident[:])
                    if k % 2 == 0:
                        nc.scalar.copy(tT[k][:, 128 * mc:128 * (mc + 1)], tp[:])
                    else:
                        nc.vector.tensor_copy(
                            tT[k][:, 128 * mc:128 * (mc + 1)], tp[:])
            return tT

        with tc.tile_pool(name="pc", bufs=1) as pc:
            h2rows = rows_from_stores(pc, 0, "h", tmajor=True)
            h2T = transpose_rows(pc, h2rows, "h")
            lhs = h2T + [ones]
            batched_xpart(pc, lhs, WIH1, XPAD1)

        recur(1, XPAD1)
        h2_batch(1)

        # ---------------- Phase E: attention + out proj ----------------
        with tc.tile_pool(name="pe", bufs=1) as pe:
            srows = rows_from_stores(pe, 1, "s", tmajor=False)
            sT = transpose_rows(pe, srows, "s")

            wint = [pe.tile([128, DS], BF16, tag=f"wi{k}", name=f"wi{k}")
                    for k in range(4)]
            for k in range(4):
                nc.sync.dma_start(wint[k][:], WINT.ap()[128 * k:128 * (k + 1), :])
            xqT = []
            for m in range(8):
                ps = gtile(m % 4, [128, R])
                for k in range(4):
                    nc.tensor.matmul(
                        ps[:], wint[k][:, 128 * m:128 * (m + 1)], sT[k][:],
                        start=(k == 0), stop=(k == 3))
                xq = pe.tile([128, R], BF16, tag=f"xq{m}", name=f"xq{m}")
                if m % 2 == 0:
                    nc.scalar.copy(xq[:], ps[:])
                else:
                    nc.vector.tensor_copy(xq[:], ps[:])
                xqT.append(xq)

            ctxT = [pe.tile([128, R], BF16, tag=f"cx{m}", name=f"cx{m}")
                    for m in range(8)]
            for b in range(BS):
                bsl = slice(T * b, T * (b + 1))
                encb = pe.tile([S, DS], BF16, tag=f"enc{b}", name=f"enc{b}")
                nc.sync.dma_start(encb[:], ENC.ap()[b])
                enctb = [pe.tile([128, S], BF16, tag=f"ect{b}{k}",
                                 name=f"ect{b}{k}") for k in range(8)]
                for k in range(8):
                    nc.sync.dma_start(
                        enctb[k][:], ENCT.ap()[b, 128 * k:128 * (k + 1), :])
                eps = gtile(2 + (b % 2), [T, S])
                for k in range(8):
                    nc.tensor.matmul(
                        eps[:], xqT[k][:, bsl], enctb[k][:],
                        start=(k == 0), stop=(k == 7))
                offsb = pe.tile([T, S], F32, tag="offs", name="offs")
                nc.sync.dma_start(offsb[:], OFFS.ap()[b])
                esb = pe.tile([T, S], F32, tag="esb", name="esb")
                nc.vector.tensor_add(esb[:], eps[:], offsb[:])
                negmax = pe.tile([T, 1], F32, tag="negmax", name="negmax")
                nc.vector.reduce_max(
                    negmax[:], esb[:], axis=mybir.AxisListType.X, negate=True)
                expE = pe.tile([T, S], F32, tag="expE", name="expE")
                den = pe.tile([T, 1], F32, tag="den", name="den")
                nc.scalar.activation(
                    expE[:], esb[:], AF.Exp, bias=negmax[:], accum_out=den[:])
                rden = pe.tile([T, 1], F32, tag="rden", name="rden")
                nc.vector.reciprocal(rden[:], den[:])
                attn = pe.tile([T, S], F32, tag="attn", name="attn")
                nc.vector.tensor_scalar_mul(attn[:], expE[:], rden[:])
                tp = psp.tile([S, T], F32, tag="tp0", name="tp0")
                nc.tensor.transpose(tp[:], attn[:], ident[0:T, 0:T])
                atsb = pe.tile([S, T], BF16, tag="atsb", name="atsb")
                nc.vector.tensor_copy(atsb[:], tp[:])
                for m in range(8):
                    psc = gtile(m % 4, [128, T])
                    nc.tensor.matmul(
                        psc[:], encb[:, 128 * m:128 * (m + 1)], atsb[:],
                        start=True, stop=True)
                    if m % 2 == 0:
                        nc.scalar.copy(ctxT[m][:, bsl], psc[:])
                    else:
                        nc.vector.tensor_copy(ctxT[m][:, bsl], psc[:])

            wout = [pe.tile([128, D], BF16, tag=f"wo{k}", name=f"wo{k}")
                    for k in range(12)]
            for k in range(12):
                nc.sync.dma_start(wout[k][:], WOUTT.ap()[128 * k:128 * (k + 1), :])
            woutb = pe.tile([1, D], BF16, tag="wo12", name="wo12")
            nc.sync.dma_start(woutb[:], WOUTT.ap()[1536:1537, :])
            outflat = OUT.ap().rearrange("b t d -> (b t) d")
            lhs_all = ctxT + sT + [ones]
            wt_all = wout + [woutb]
            for mc in range(2):
                msl = slice(128 * mc, 128 * (mc + 1))
                ps = gtile(mc, [128, D])
                for k in range(13):
                    nc.tensor.matmul(
                        ps[:], lhs_all[k][:, msl], wt_all[k][:],
                        start=(k == 0), stop=(k == 12))
                osb = pe.tile([128, D], F32, tag=f"osb{mc}", name=f"osb{mc}")
                nc.scalar.activation(osb[:], ps[:], AF.Tanh)
                nc.sync.dma_start(outflat[msl, :], osb[:])

    nc.compile()
    return nc


def assemble(results):
    full = np.concatenate([r["out"] for r in results], axis=0)  # [B, T, D]
    outs = full.transpose(1, 0, 2)                              # [T, B, D]
    return np.ascontiguousarray(outs.reshape(-1, D).reshape(-1, T, D))


_nc_cache = None


def kernel(**inputs):
    global _nc_cache
    in_maps = host_prep(inputs)
    if _nc_cache is None:
        _nc_cache = build_program()
    res = run_bass_kernel_spmd(_nc_cache, in_maps, list(range(NCORES)))
    return assemble(res.results)


# revision 26
# speedup vs baseline: 1.3901x; 1.3901x over previous
"""Trainium2 Bass kernel for nn_Decoder_46042049413334.

Buggy 2-layer LSTM decoder with attention (B=32, T=64, S=128, D=512).

v3: layer-interleaved col-tiled recurrence. Both layers advance together
each step t: L0's gates psum (bank A) and L1's (bank B) are produced by
4-way column-tiled matmul streams; L1's xpart (h2_0 @ W_ih1 + bias) is
computed per step from the in-SBUF transposed h2_0, so no xpart-1 GEMM
phase, no h2 DRAM roundtrip, and the PE stays busy (HAM warm) during the
elementwise windows. h2/s transposes accumulate into persistent SBUF
tiles (h2T0, s2T) consumed directly by the attention phase.

Gates col layout per 128-d-slice c: [i | f | g | o], g pre-scaled by 2
so tanh(g) = 2*sig(2g) - 1 comes from one 384-wide sigmoid + DVE ops.
Phase A rows are t-major (r = 4t + b); phase E is b-major as usual.
"""
import numpy as np
import ml_dtypes
from contextlib import ExitStack

import concourse.bass as bass
import concourse.bacc as bacc
import concourse.tile as tile
from concourse import mybir, masks
from concourse.bass_utils import run_bass_kernel_spmd

F32 = mybir.dt.float32
BF16 = mybir.dt.bfloat16
AF = mybir.ActivationFunctionType
ALU = mybir.AluOpType
NPBF = ml_dtypes.bfloat16

B, T, S, D, L, V = 32, 64, 128, 512, 2, 32000
G = 4 * D        # 2048
DS = 2 * D       # 1024
NCORES = 8
BS = B // NCORES  # 4
R = BS * T        # 256 rows per core


# ---------------------------------------------------------------- host side

def _gate_perm():
    """New gate col n' = 512c + 128s + dd  <-  orig = 512s + 128c + dd."""
    idx = np.arange(G)
    c, rem = idx // 512, idx % 512
    s, dd = rem // 128, rem % 128
    return 512 * s + 128 * c + dd


def _g_scale():
    idx = np.arange(G)
    s = (idx % 512) // 128
    return np.where(s == 2, 2.0, 1.0)


def host_prep(inputs):
    perm = _gate_perm()
    gsc = _g_scale()
    tokens = np.asarray(inputs["prev_tgt_tokens"])
    embed = np.asarray(inputs["embed"], dtype=np.float32)
    enc = np.asarray(inputs["encoder_out"], dtype=np.float32)
    mask = np.asarray(inputs["src_mask"])
    hid = np.asarray(inputs["hiddens"], dtype=np.float32)
    cells = np.asarray(inputs["cells"], dtype=np.float32)
    W_ih = np.asarray(inputs["W_ih"], dtype=np.float32)
    W_hh = np.asarray(inputs["W_hh"], dtype=np.float32)
    b_ih = np.asarray(inputs["b_ih"], dtype=np.float32)
    b_hh = np.asarray(inputs["b_hh"], dtype=np.float32)
    W_in = np.asarray(inputs["W_in"], dtype=np.float32)
    b_in = np.asarray(inputs["b_in"], dtype=np.float32)
    W_out = np.asarray(inputs["W_out"], dtype=np.float32)
    b_out = np.asarray(inputs["b_out"], dtype=np.float32)

    def bf(x):
        return np.ascontiguousarray(x, dtype=NPBF)

    WIH = []
    WHH = []
    BIASP = []
    for l in range(L):
        wihT = W_ih[l].T[:, perm] * gsc[None, :]
        biasrow = (b_ih[l] + b_hh[l])[perm] * gsc
        WIH.append(bf(np.concatenate([wihT, biasrow[None, :]], 0)))
        WHH.append(bf(W_hh[l].T[:, perm] * gsc[None, :]))    # [512, 2048]
        BIASP.append(biasrow)
    WINT = bf(W_in.T)                                        # [512, 1024]
    WOUTT = bf(np.concatenate([W_out.T, b_out[None, :]], 0))  # [1537, 512]

    # bias4 for L1: rows 32c+q all hold bias slice c
    bias4 = np.zeros((128, 512), np.float32)
    for c in range(4):
        bias4[32 * c:32 * c + 32, :] = BIASP[1][512 * c:512 * (c + 1)][None, :]
    bias4 = bf(bias4)

    # I16[k, 4j+b] = 1 iff k == 32j + b (f32 for c2 transposes, bf16 for h2)
    I16 = np.zeros((128, 16), np.float32)
    for j in range(4):
        for b in range(4):
            I16[32 * j + b, 4 * j + b] = 1.0
    # IWC[c, k, 32*tl + m] = 1 iff m < 4 and k == 32c + 4*tl + m
    IWC = np.zeros((4, 128, 256), np.float32)
    for c in range(4):
        for tl in range(8):
            for mm in range(4):
                IWC[c, 32 * c + 4 * tl + mm, 32 * tl + mm] = 1.0
    IWC = bf(IWC)

    in_maps = []
    for core in range(NCORES):
        bsl = slice(core * BS, (core + 1) * BS)
        xe = embed[tokens[bsl]]                              # [BS, T, D]
        Xaug = np.concatenate(
            [xe.transpose(1, 0, 2).reshape(R, D),
             np.ones((R, 1), np.float32)], axis=1)           # t-major rows
        XT0 = bf(Xaug.T)                                     # [513, 256]
        enc_c = np.ascontiguousarray(enc[bsl])               # [BS, 128, 1024]
        encT_c = np.swapaxes(enc_c, 1, 2)                    # [BS, 1024, 128]
        offs = np.einsum("bsd,d->bs", enc_c, b_in) + np.where(mask[bsl], -1e9, 0.0)
        offs_rep = np.ascontiguousarray(
            np.broadcast_to(offs[:, None, :], (BS, T, S)), dtype=np.float32)
        hl = hid[:, bsl]                                     # [L, 4, 512]
        c2t0 = np.zeros((L, 128, 16), np.float32)
        for ko in range(4):
            for b in range(4):
                c2t0[:, :, 4 * ko + b] = hl[:, b, 128 * ko:128 * ko + 128]
        cl = cells[:, bsl]                                   # [L, 4, 512]
        c0g = np.zeros((L, 128, 128), np.float32)
        for q in range(4):
            for b in range(4):
                c0g[:, 32 * q + b, :] = cl[:, b, 128 * q:128 * q + 128]
        in_maps.append({
            "xt0": XT0,
            "wih0": WIH[0], "whh0": WHH[0],
            "wih1": WIH[1], "whh1": WHH[1],
            "wint": WINT, "woutt": WOUTT,
            "enc": bf(enc_c), "enct": bf(encT_c), "offs": offs_rep,
            "c2t0": bf(c2t0), "c0g": bf(c0g),
            "i16": I16, "i16b": bf(I16), "iw": IWC, "bias4": bias4,
            "ones1": np.ones((1, R), NPBF),
        })
    return in_maps


# ------------------------------------------------------------- device build

def build_program():
    nc = bacc.Bacc("TRN2", target_bir_lowering=False, debug=False)

    XT0 = nc.dram_tensor("xt0", [513, R], BF16, kind="ExternalInput")
    WIH0 = nc.dram_tensor("wih0", [513, G], BF16, kind="ExternalInput")
    WHH0 = nc.dram_tensor("whh0", [D, G], BF16, kind="ExternalInput")
    WIH1 = nc.dram_tensor("wih1", [513, G], BF16, kind="ExternalInput")
    WHH1 = nc.dram_tensor("whh1", [D, G], BF16, kind="ExternalInput")
    WINT = nc.dram_tensor("wint", [D, DS], BF16, kind="ExternalInput")
    WOUTT = nc.dram_tensor("woutt", [DS + D + 1, D], BF16, kind="ExternalInput")
    ENC = nc.dram_tensor("enc", [BS, S, DS], BF16, kind="ExternalInput")
    ENCT = nc.dram_tensor("enct", [BS, DS, S], BF16, kind="ExternalInput")
    OFFS = nc.dram_tensor("offs", [BS, T, S], F32, kind="ExternalInput")
    C2T0 = nc.dram_tensor("c2t0", [L, 128, 16], BF16, kind="ExternalInput")
    C0G = nc.dram_tensor("c0g", [L, 128, 128], BF16, kind="ExternalInput")
    I16T = nc.dram_tensor("i16", [128, 16], F32, kind="ExternalInput")
    I16BT = nc.dram_tensor("i16b", [128, 16], BF16, kind="ExternalInput")
    IWT = nc.dram_tensor("iw", [4, 128, 256], BF16, kind="ExternalInput")
    BIAS4T = nc.dram_tensor("bias4", [128, 512], BF16, kind="ExternalInput")
    ONES1 = nc.dram_tensor("ones1", [1, R], BF16, kind="ExternalInput")
    OUT = nc.dram_tensor("out", [BS, T, D], F32, kind="ExternalOutput")

    XPAD0 = nc.dram_tensor("xpad0", [T // 8, 128, 512], BF16, kind="Internal")

    with tile.TileContext(nc) as tc, ExitStack() as ctx:
        cpool = ctx.enter_context(tc.tile_pool(name="const", bufs=1))
        ident = cpool.tile([128, 128], F32)
        masks.make_identity(nc, ident[:])
        ones = cpool.tile([1, R], BF16)
        nc.sync.dma_start(ones[:], ONES1.ap())
        i16 = cpool.tile([128, 16], F32)
        nc.sync.dma_start(i16[:], I16T.ap())
        i16b = cpool.tile([128, 16], BF16)
        nc.sync.dma_start(i16b[:], I16BT.ap())
        iw = [cpool.tile([128, 256], BF16, tag=f"iw{c}", name=f"iw{c}")
              for c in range(4)]
        for c in range(4):
            nc.sync.dma_start(iw[c][:], IWT.ap()[c])
        bias4 = cpool.tile([128, 512], BF16)
        nc.sync.dma_start(bias4[:], BIAS4T.ap())

        psp = ctx.enter_context(tc.tile_pool(name="ps", bufs=1, space="PSUM"))

        def gtile(idx, shape):
            return psp.tile(shape, F32, tag=f"g{idx % 2}", name=f"g{idx % 2}",
                            bufs=1)

        # persistent weight tiles: whh0/whh1/wih1 ko-chunks
        wbpool = ctx.enter_context(tc.tile_pool(name="wb", bufs=1))
        wtiles = {}
        for nm, Wd in (("whh0", WHH0), ("whh1", WHH1), ("wih1", WIH1)):
            tt = [wbpool.tile([128, G], BF16, tag=f"{nm}k{k}",
                              name=f"{nm}k{k}") for k in range(4)]
            for k in range(4):
                nc.sync.dma_start(tt[k][:], Wd.ap()[128 * k:128 * (k + 1), :])
            wtiles[nm] = tt

        # ---------------- Phase A: xpart0 -> XPAD0 ----------------
        with tc.tile_pool(name="pa", bufs=1) as pa:
            xt = [pa.tile([128, R], BF16, tag=f"xt{k}", name=f"xt{k}")
                  for k in range(4)]
            xt.append(pa.tile([1, R], BF16, tag="xt4", name="xt4"))
            for k in range(4):
                nc.sync.dma_start(xt[k][:], XT0.ap()[128 * k:128 * (k + 1), :])
            nc.sync.dma_start(xt[4][:], XT0.ap()[512:513, :])
            wt = [pa.tile([128, G], BF16, tag=f"wk{k}", name=f"wk{k}")
                  for k in range(4)]
            wt.append(pa.tile([1, G], BF16, tag="wk4", name="wk4"))
            for k in range(4):
                nc.sync.dma_start(wt[k][:], WIH0.ap()[128 * k:128 * (k + 1), :])
            nc.sync.dma_start(wt[4][:], WIH0.ap()[512:513, :])
            for mc in range(2):
                for nb in range(4):
                    ps = gtile(nb, [128, 512])
                    for k in range(5):
                        nc.tensor.matmul(
                            ps[:],
                            xt[k][:, 128 * mc:128 * (mc + 1)],
                            wt[k][:, 512 * nb:512 * (nb + 1)],
                            start=(k == 0), stop=(k == 4))
                    sb = pa.tile([128, 512], BF16, tag=f"stg{nb}",
                                 name=f"stg{nb}")
                    nc.scalar.copy(sb[:], ps[:])
                    nc.sync.dma_start(
                        XPAD0.ap()[4 * mc:4 * mc + 4,
                                   32 * nb:32 * nb + 32, :],
                        sb[:])

        # ---------------- Interleaved recurrence ----------------
        rp = ctx.enter_context(tc.tile_pool(name="rr", bufs=1))
        c0g = {}
        c2t = {}
        for l in range(L):
            c0g[l] = rp.tile([128, 128], BF16, tag=f"c0g{l}", name=f"c0g{l}")
            nc.sync.dma_start(c0g[l][:], C0G.ap()[l])
            c2t[l] = rp.tile([128, 16], BF16, tag=f"c2t{l}", name=f"c2t{l}",
                             bufs=2)
            nc.sync.dma_start(c2t[l][:], C2T0.ap()[l])
        h2T0 = rp.tile([128, 1024], BF16, tag="h2T0", name="h2T0")
        s2T = rp.tile([128, 1024], BF16, tag="s2T", name="s2T")

        xp_tiles = {}

        def xp_load(g8):
            xp = rp.tile([128, 512], BF16, tag="xp", name="xp", bufs=3)
            nc.sync.dma_start(xp[:], XPAD0.ap()[g8])
            xp_tiles[g8] = xp

        gates = {}

        def xp0_mm(t):
            ps = psp.tile([128, 512], F32, tag="ra", name="ra", bufs=2)
            gates[t] = ps
            tl = t % 8
            for c in range(4):
                nc.tensor.matmul(
                    ps[32 * c:32 * c + 32, :],
                    iw[c][:, 32 * tl:32 * tl + 32],
                    xp_tiles[t // 8][:],
                    start=True, stop=False,
                    tile_position=(0, 32 * c), skip_group_check=True)

        def whh_mms(ps, state, wname, stop_last):
            tt = wtiles[wname]
            for ko in range(4):
                for c in range(4):
                    nc.tensor.matmul(
                        ps[32 * c:32 * c + 4, :],
                        state[:, 4 * ko:4 * ko + 4],
                        tt[ko][:, 512 * c:512 * (c + 1)],
                        start=False, stop=(stop_last and ko == 3),
                        tile_position=(0, 32 * c), skip_group_check=True)

        def bias_mm(ps):
            for c in range(4):
                nc.tensor.matmul(
                    ps[32 * c:32 * c + 32, :],
                    iw[c][:, 0:32], bias4[:],
                    start=True, stop=False,
                    tile_position=(0, 32 * c), skip_group_check=True)

        def xp1_mms(ps, t):
            tt = wtiles["wih1"]
            for ko in range(4):
                lhs = h2T0[:, 16 * t + 4 * ko:16 * t + 4 * ko + 4]
                for c in range(4):
                    nc.tensor.matmul(
                        ps[32 * c:32 * c + 4, :],
                        lhs,
                        tt[ko][:, 512 * c:512 * (c + 1)],
                        start=False, stop=(ko == 3),
                        tile_position=(0, 32 * c), skip_group_check=True)

        def lchain(l, ps, t):
            """sigmoid + c2/h2 chain; returns (c2g f32, h2b bf16)."""
            sg = rp.tile([128, 384], F32, tag=f"sg{l}", name=f"sg{l}", bufs=2)
            nc.scalar.activation(sg[:], ps[:, 0:384], AF.Sigmoid)
            so = rp.tile([128, 128], F32, tag=f"so{l}", name=f"so{l}", bufs=2)
            nc.scalar.activation(so[:], ps[:, 384:512], AF.Sigmoid)
            u = rp.tile([128, 128], F32, tag=f"u{l}", name=f"u{l}", bufs=2)
            nc.vector.scalar_tensor_tensor(
                u[:], sg[:, 256:384], 2.0, sg[:, 0:128],
                op0=ALU.mult, op1=ALU.mult)
            v = rp.tile([128, 128], F32, tag=f"v{l}", name=f"v{l}", bufs=2)
            nc.gpsimd.tensor_mul(v[:], sg[:, 128:256], c0g[l][:])
            w = rp.tile([128, 128], F32, tag=f"w{l}", name=f"w{l}", bufs=2)
            nc.vector.tensor_sub(w[:], u[:], sg[:, 0:128])
            c2g = rp.tile([128, 128], F32, tag=f"c2g{l}", name=f"c2g{l}",
                          bufs=2)
            nc.vector.tensor_add(c2g[:], w[:], v[:])
            s2c = rp.tile([128, 128], F32, tag=f"s2c{l}", name=f"s2c{l}",
                          bufs=2)
            nc.scalar.activation(s2c[:], c2g[:], AF.Sigmoid, scale=2.0)
            p = rp.tile([128, 128], F32, tag=f"p{l}", name=f"p{l}", bufs=2)
            nc.gpsimd.tensor_mul(p[:], so[:], s2c[:])
            h2b = rp.tile([128, 128], BF16, tag=f"h2b{l}", name=f"h2b{l}",
                          bufs=2)
            nc.vector.scalar_tensor_tensor(
                h2b[:], p[:], 2.0, so[:], op0=ALU.mult, op1=ALU.subtract)
            return c2g, h2b

        def transp(src, ident16):
            tps = psp.tile([128, 16], F32, tag="rt", name="rt", bufs=1)
            for cp in range(4):
                nc.tensor.matmul(
                    tps[32 * cp:32 * cp + 32, :],
                    src[:, 32 * cp:32 * cp + 32], ident16[:],
                    start=True, stop=True,
                    tile_position=(0, 32 * cp), skip_group_check=True)
            return tps

        def cast_c2t(l, tps):
            nt = rp.tile([128, 16], BF16, tag=f"c2t{l}", name=f"c2t{l}",
                         bufs=2)
            nc.vector.tensor_copy(nt[:], tps[:])
            c2t[l] = nt

        xp_load(0)
        xp_load(1)
        xp0_mm(0)

        pend1 = None  # (c2g_1, h2b_1, t) awaiting transposes
        for t in range(T):
            ps0 = gates.pop(t)
            # L0 W-pass
            whh_mms(ps0, c2t[0][:], "whh0", stop_last=True)
            if t + 1 < T:
                xp0_mm(t + 1)
            # L1 transposes from t-1 (fills part of L0's chain window)
            if pend1 is not None:
                c2g1p, h2b1p, tp_ = pend1
                tps = transp(c2g1p, i16)
                cast_c2t(1, tps)
                tpsh = transp(h2b1p, i16b)
                nc.vector.tensor_copy(s2T[:, 16 * tp_:16 * tp_ + 16], tpsh[:])
            # L1 gates psum: bias + whh1 early (PE fill), xp1 later
            ps1 = psp.tile([128, 512], F32, tag="rb", name="rb", bufs=2)
            bias_mm(ps1)
            whh_mms(ps1, c2t[1][:], "whh1", stop_last=False)
            # L0 chain
            c2g0, h2b0 = lchain(0, ps0, t)
            if t + 1 < T:
                tps = transp(c2g0, i16)
                cast_c2t(0, tps)
            tpsh = transp(h2b0, i16b)
            nc.vector.tensor_copy(h2T0[:, 16 * t:16 * t + 16], tpsh[:])
            # L1 xpart from h2_0(t)
            xp1_mms(ps1, t)
            # L1 chain
            c2g1, h2b1 = lchain(1, ps1, t)
            pend1 = (c2g1, h2b1, t)
            if (t + 2) % 8 == 0 and t + 2 < T - 8:
                xp_load((t + 2) // 8 + 1)
        # final L1 transpose (s for t = T-1)
        c2g1p, h2b1p, tp_ = pend1
        tpsh = transp(h2b1p, i16b)
        nc.vector.tensor_copy(s2T[:, 16 * tp_:16 * tp_ + 16], tpsh[:])

        # ---------------- Phase E: attention + out proj ----------------
        with tc.tile_pool(name="pe", bufs=1) as pe:
            # sT[k][:, 64b + t] = s2T[:, 16t + 4k + b]  (b-major cols)
            s2v = s2T[:].rearrange("p (t k b) -> p k b t", t=T, k=4, b=4)
            sT = [pe.tile([128, R], BF16, tag=f"sT{k}", name=f"sT{k}")
                  for k in range(4)]
            for k in range(4):
                nc.vector.tensor_copy(sT[k][:], s2v[:, k])

            wint = [pe.tile([128, DS], BF16, tag=f"wi{k}", name=f"wi{k}")
                    for k in range(4)]
            for k in range(4):
                nc.sync.dma_start(wint[k][:], WINT.ap()[128 * k:128 * (k + 1), :])
            xqT = []
            for m in range(8):
                ps = gtile(m, [128, R])
                for k in range(4):
                    nc.tensor.matmul(
                        ps[:], wint[k][:, 128 * m:128 * (m + 1)], sT[k][:],
                        start=(k == 0), stop=(k == 3))
                xq = pe.tile([128, R], BF16, tag=f"xq{m}", name=f"xq{m}")
                if m % 2 == 0:
                    nc.scalar.copy(xq[:], ps[:])
                else:
                    nc.vector.tensor_copy(xq[:], ps[:])
                xqT.append(xq)

            ctxT = [pe.tile([128, R], BF16, tag=f"cx{m}", name=f"cx{m}")
                    for m in range(8)]
            for b in range(BS):
                bsl = slice(T * b, T * (b + 1))
                encb = pe.tile([S, DS], BF16, tag=f"enc{b}", name=f"enc{b}")
                nc.sync.dma_start(encb[:], ENC.ap()[b])
                enctb = [pe.tile([128, S], BF16, tag=f"ect{b}{k}",
                                 name=f"ect{b}{k}") for k in range(8)]
                for k in range(8):
                    nc.sync.dma_start(
                        enctb[k][:], ENCT.ap()[b, 128 * k:128 * (k + 1), :])
                eps = gtile(b, [T, S])
                for k in range(8):
                    nc.tensor.matmul(
                        eps[:], xqT[k][:, bsl], enctb[k][:],
                        start=(k == 0), stop=(k == 7))
                offsb = pe.tile([T, S], F32, tag="offs", name="offs")
                nc.sync.dma_start(offsb[:], OFFS.ap()[b])
                esb = pe.tile([T, S], F32, tag="esb", name="esb")
                nc.vector.tensor_add(esb[:], eps[:], offsb[:])
                negmax = pe.tile([T, 1], F32, tag="negmax", name="negmax")
                nc.vector.reduce_max(
                    negmax[:], esb[:], axis=mybir.AxisListType.X, negate=True)
                expE = pe.tile([T, S], F32, tag="expE", name="expE")
                den = pe.tile([T, 1], F32, tag="den", name="den")
                nc.scalar.activation(
                    expE[:], esb[:], AF.Exp, bias=negmax[:], accum_out=den[:])
                rden = pe.tile([T, 1], F32, tag="rden", name="rden")
                nc.vector.reciprocal(rden[:], den[:])
                attn = pe.tile([T, S], F32, tag="attn", name="attn")
                nc.vector.tensor_scalar_mul(attn[:], expE[:], rden[:])
                tp = psp.tile([S, T], F32, tag="tp0", name="tp0")
                nc.tensor.transpose(tp[:], attn[:], ident[0:T, 0:T])
                atsb = pe.tile([S, T], BF16, tag="atsb", name="atsb")
                nc.vector.tensor_copy(atsb[:], tp[:])
                for m in range(8):
                    psc = gtile(m, [128, T])
                    nc.tensor.matmul(
                        psc[:], encb[:, 128 * m:128 * (m + 1)], atsb[:],
                        start=True, stop=True)
                    if m % 2 == 0:
                        nc.scalar.copy(ctxT[m][:, bsl], psc[:])
                    else:
                        nc.vector.tensor_copy(ctxT[m][:, bsl], psc[:])

            wout = [pe.tile([128, D], BF16, tag=f"wo{k}", name=f"wo{k}")
                    for k in range(12)]
            for k in range(12):
                nc.sync.dma_start(wout[k][:], WOUTT.ap()[128 * k:128 * (k + 1), :])
            woutb = pe.tile([1, D], BF16, tag="wo12", name="wo12")
            nc.sync.dma_start(woutb[:], WOUTT.ap()[1536:1537, :])
            outflat = OUT.ap().rearrange("b t d -> (b t) d")
            lhs_all = ctxT + sT + [ones]
            wt_all = wout + [woutb]
            for mc in range(2):
                msl = slice(128 * mc, 128 * (mc + 1))
                ps = gtile(mc, [128, D])
                for k in range(13):
                    nc.tensor.matmul(
                        ps[:], lhs_all[k][:, msl], wt_all[k][:],
                        start=(k == 0), stop=(k == 12))
                osb = pe.tile([128, D], F32, tag=f"osb{mc}", name=f"osb{mc}")
                nc.scalar.activation(osb[:], ps[:], AF.Tanh)
                nc.sync.dma_start(outflat[msl, :], osb[:])

    nc.compile()
    return nc


def assemble(results):
    full = np.concatenate([r["out"] for r in results], axis=0)  # [B, T, D]
    outs = full.transpose(1, 0, 2)                              # [T, B, D]
    return np.ascontiguousarray(outs.reshape(-1, D).reshape(-1, T, D))


_nc_cache = None


def kernel(**inputs):
    global _nc_cache
    in_maps = host_prep(inputs)
    if _nc_cache is None:
        _nc_cache = build_program()
    res = run_bass_kernel_spmd(_nc_cache, in_maps, list(range(NCORES)))
    return assemble(res.results)
